# revision 1
# baseline (speedup 1.0000x reference)
"""Trainium2 Bass kernel for CausalSelfAttention (B=4, T=2048, C=768, H=6, D=128)
with RoPE + QK-RMSNorm.

Sharding: 8 cores = batch(4) x head-group(2, 3 heads each). Each core:
  - computes Q^T,K^T in (D, T) layout and V in (T, D) layout for its 3 heads
  - RoPE + RMSNorm on Q/K (partition-dim reductions via ones-matmul on PE)
  - causal attention with scores computed transposed (S^T: T_k on partitions,
    T_q on free dim) so softmax denom + AV matmuls need no transposes
  - partial c_proj over its 384 input channels
Host sums the two head-group partials per batch.
"""

import numpy as np

_B, _T, _C, _H, _D = 4, 2048, 768, 6, 128
_HPG = 3            # heads per group
_HD = _HPG * _D     # 384, per-group head dims
_NT = 4             # T tiles of 512
_TW = 512           # tile width (T_q)
_NKC = _T // 128    # 16 k-chunks of 128
_NCB = _C // 128    # 6 c_in chunks
_EPS = 1e-15

_cached = {}


def _build_nc():
    from contextlib import ExitStack
    from concourse import bacc, tile, mybir

    f32 = mybir.dt.float32
    f32r = mybir.dt.float32r
    Act = mybir.ActivationFunctionType
    Op = mybir.AluOpType

    nc = bacc.Bacc("TRN2", target_bir_lowering=False, debug=False)

    xT = nc.dram_tensor("xT", (_C, _T), f32r, kind="ExternalInput").ap()
    wq = nc.dram_tensor("wq", (_C, _HD), f32r, kind="ExternalInput").ap()
    wk = nc.dram_tensor("wk", (_C, _HD), f32r, kind="ExternalInput").ap()
    wv = nc.dram_tensor("wv", (_C, _HD), f32r, kind="ExternalInput").ap()
    wo = nc.dram_tensor("wo", (_HD, _C), f32r, kind="ExternalInput").ap()
    cc = nc.dram_tensor("cc", (128, _T), f32r, kind="ExternalInput").ap()
    ss = nc.dram_tensor("ss", (128, _T), f32r, kind="ExternalInput").ap()
    tri = nc.dram_tensor("tri", (128, 128), f32r, kind="ExternalInput").ap()
    ones = nc.dram_tensor("ones", (128, 128), f32r, kind="ExternalInput").ap()
    perm = nc.dram_tensor("perm", (128, 128), f32r, kind="ExternalInput").ap()
    out = nc.dram_tensor("out", (_T, _C), f32, kind="ExternalOutput").ap()

    with tile.TileContext(nc) as tc, ExitStack() as ctx, \
            nc.allow_low_precision(reason="f32r tiles carry full fp32 bits; PE rounds at ingest"):
        # --- pools ---
        pc = ctx.enter_context(tc.tile_pool(name="pc", bufs=1))
        pg = ctx.enter_context(tc.tile_pool(name="pg", bufs=2))         # Q tile scratch
        pa = ctx.enter_context(tc.tile_pool(name="pa", bufs=4))         # A chunks
        pz = ctx.enter_context(tc.tile_pool(name="pz", bufs=6))         # Z chunks
        psm = ctx.enter_context(tc.tile_pool(name="psm", bufs=2))       # small (1,512)/(128,512)
        pob = ctx.enter_context(tc.tile_pool(name="pob", bufs=2))       # out staging
        # psum pools (8 banks total)
        ppq = ctx.enter_context(tc.tile_pool(name="ppq", bufs=2, space="PSUM"))
        pps = ctx.enter_context(tc.tile_pool(name="pps", bufs=2, space="PSUM"))
        ppo = ctx.enter_context(tc.tile_pool(name="ppo", bufs=2, space="PSUM"))
        ppd = ctx.enter_context(tc.tile_pool(name="ppd", bufs=1, space="PSUM"))
        ppm = ctx.enter_context(tc.tile_pool(name="ppm", bufs=1, space="PSUM"))

        # --- constants / inputs resident in SBUF ---
        # load order matters: wk + xT feed the first PE work (K-projections);
        # cc/ss are not needed until rope, wq not until phase B, wo until c_proj
        t_wq, t_wk, t_wv = [], [], []
        for c in range(_NCB):
            t = pc.tile([128, _HD], f32r, tag=f"wk{c}", name=f"wk{c}",
                        padded_shape=[128, _TW])
            nc.sync.dma_start(t[:], wk[c * 128:(c + 1) * 128, :])
            t_wk.append(t)
        t_xt = []
        for c in range(_NCB):
            t = pc.tile([128, _T], f32r, tag=f"xt{c}", name=f"xt{c}")
            nc.sync.dma_start(t[:], xT[c * 128:(c + 1) * 128, :])
            t_xt.append(t)
        for c in range(_NCB):
            t = pc.tile([128, _HD], f32r, tag=f"wv{c}", name=f"wv{c}",
                        padded_shape=[128, _TW])
            nc.sync.dma_start(t[:], wv[c * 128:(c + 1) * 128, :])
            t_wv.append(t)
        t_cc = pc.tile([128, _T], f32r, tag="cc")
        t_ss = pc.tile([128, _T], f32r, tag="ss")
        nc.sync.dma_start(t_cc[:], cc[:])
        nc.sync.dma_start(t_ss[:], ss[:])
        for c in range(_NCB):
            t = pc.tile([128, _HD], f32r, tag=f"wq{c}", name=f"wq{c}")
            nc.sync.dma_start(t[:], wq[c * 128:(c + 1) * 128, :])
            t_wq.append(t)
        t_tri = pc.tile([128, 128], f32r, tag="tri")
        t_ones = pc.tile([128, 128], f32r, tag="ones")
        t_perm = pc.tile([128, 128], f32r, tag="perm")
        nc.sync.dma_start(t_tri[:], tri[:])
        nc.sync.dma_start(t_ones[:], ones[:])
        nc.sync.dma_start(t_perm[:], perm[:])
        t_ones_col = t_ones[:, 0:1]
        t_ones_row = t_ones[0:1, :]
        t_eps = pc.tile([128, 1], f32, tag="eps")
        nc.gpsimd.memset(t_eps[:], _EPS)
        t_wo = []
        for c in range(_HPG):
            t = pc.tile([128, _C], f32r, tag=f"wo{c}", name=f"wo{c}")
            nc.sync.dma_start(t[:], wo[c * 128:(c + 1) * 128, :])
            t_wo.append(t)

        # persistent K^T (post rope+norm) per head, and V blocks
        t_kn = [pc.tile([128, _T], f32r, tag=f"kn{h}", name=f"kn{h}") for h in range(_HPG)]
        t_v = [pc.tile([128, _HD], f32r, tag=f"v{tb}", name=f"v{tb}") for tb in range(_NKC)]

        def rope_part(dst_ap, col0):
            """In-place RoPE on dst_ap (128, 512)."""
            csl = slice(col0, col0 + _TW)
            p_sw = pps.tile([128, _TW], f32, tag="ps", name="p_sw")
            nc.tensor.matmul(p_sw[:], t_perm[:], dst_ap, start=True, stop=True)
            t_sw = pg.tile([128, _TW], f32r, tag="sw512", name="sw512", bufs=3)
            nc.vector.tensor_mul(dst_ap, dst_ap, t_cc[:, csl])
            nc.vector.tensor_mul(t_sw[:], p_sw[:], t_ss[:, csl])
            nc.vector.tensor_add(dst_ap, dst_ap, t_sw[:])

        def norm_pre(dst_ap, bc_pool, bc_tag, ms_on_act=True):
            """Square + partition-sum + broadcast; returns bcast psum."""
            t_sq = pg.tile([128, _TW], f32r, tag="sq512", name="sq512", bufs=3)
            nc.vector.tensor_mul(t_sq[:], dst_ap, dst_ap)
            p_ms = ppm.tile([1, _TW], f32, tag="pms", name="p_ms")
            nc.tensor.matmul(p_ms[:], t_ones_col, t_sq[:], start=True, stop=True)
            t_ms = psm.tile([1, _TW], f32r, tag="ms", name="t_ms", bufs=3)
            if ms_on_act:
                nc.scalar.copy(t_ms[:], p_ms[:])
            else:
                nc.vector.tensor_copy(t_ms[:], p_ms[:])
            p_bc = bc_pool.tile([128, _TW], f32, tag=bc_tag, name="p_bc")
            nc.tensor.matmul(p_bc[:], t_ones_row, t_ms[:], start=True, stop=True)
            return p_bc

        def norm_post(dst_ap, p_bc):
            """sqrt -> reciprocal -> scale, in place on dst_ap."""
            t_sd = psm.tile([128, _TW], f32r, tag="sd", name="t_sd", bufs=3)
            nc.scalar.activation(t_sd[:], p_bc[:], Act.Sqrt,
                                 bias=t_eps[:], scale=1.0 / 128.0)
            nc.vector.reciprocal(t_sd[:], t_sd[:])
            nc.vector.tensor_mul(dst_ap, dst_ap, t_sd[:])

        # one bcast-psum route per head so three chains can be in flight
        _bc_routes = [(pps, "ps"), (ppq, "pq"), (ppd, "pd")]

        def rope_norm(dst_ap, tw, col0):
            rope_part(dst_ap, col0)
            norm_post(dst_ap, norm_pre(dst_ap, pps, "ps"))

        # ---------------- Phase A: K^T (rope+norm) and V ----------------
        for i in range(_NT):
            isl = slice(i * _TW, (i + 1) * _TW)
            for h in range(_HPG):
                hsl = slice(h * 128, (h + 1) * 128)
                p_k = pps.tile([128, _TW], f32, tag="ps")
                for c in range(_NCB):
                    nc.tensor.matmul(p_k[:], t_wk[c][:, hsl], t_xt[c][:, isl],
                                     start=(c == 0), stop=(c == _NCB - 1))
                nc.scalar.copy(t_kn[h][:, isl], p_k[:])
        # V-projs emitted here: independent PE work that fills the gaps in
        # the serial rope+norm chains below
        for tb in range(_NKC):
            bsl = slice(tb * 128, (tb + 1) * 128)
            p_v = ppo.tile([128, _HD], f32, tag="po")
            for c in range(_NCB):
                nc.tensor.matmul(p_v[:], t_xt[c][:, bsl], t_wv[c][:],
                                 start=(c == 0), stop=(c == _NCB - 1))
            nc.scalar.copy(t_v[tb][:], p_v[:])
        # stage-batched across heads: three chains in flight, each using its
        # own bcast-psum pool (ppq/ppd are otherwise idle in phase A)
        for i in range(_NT):
            isl = slice(i * _TW, (i + 1) * _TW)
            for h in range(_HPG):
                rope_part(t_kn[h][:, isl], i * _TW)
            bcs = []
            for h in range(_HPG):
                pool, tag = _bc_routes[h]
                bcs.append(norm_pre(t_kn[h][:, isl], pool, tag))
            for h in range(_HPG):
                norm_post(t_kn[h][:, isl], bcs[h])

        # ---------------- Phase B: per T_q tile ----------------
        a_ctr = [0]

        def q_chain(qt, h):
            qsl = slice(qt * _TW, (qt + 1) * _TW)
            hsl = slice(h * 128, (h + 1) * 128)
            p_q = ppq.tile([128, _TW], f32, tag="pq", name="p_q")
            for c in range(_NCB):
                nc.tensor.matmul(p_q[:], t_wq[c][:, hsl], t_xt[c][:, qsl],
                                 start=(c == 0), stop=(c == _NCB - 1))
            t_g = pg.tile([128, _TW], f32r, tag="g", name="g", bufs=7)
            nc.vector.tensor_copy(t_g[:], p_q[:])
            rope_part(t_g[:], qt * _TW)
            pool, tag = _bc_routes[h] if h < 2 else (pps, "ps")
            norm_post(t_g[:], norm_pre(t_g[:], pool, tag, ms_on_act=False))
            return t_g

        def attention(qt, h, t_g):
            """Causal attention for one (T_q tile, head). The den/AV matmuls
            are emitted LOOKAHEAD chunks behind the S/exp pair: the PE stream
            is in-order, so den(kc) stalls on exp(kc) unless later S-matmuls
            are issued first."""
            hsl = slice(h * 128, (h + 1) * 128)
            nchunk = 4 * qt + 4
            LOOKAHEAD = 3
            p_den = ppd.tile([1, _TW], f32, tag="pd", name="p_den")
            p_o = ppo.tile([128, _TW], f32, tag="po", name="p_o")
            a_tiles = {}

            def emit_s(kc):
                roff = 0 if kc < 4 * qt else (kc - 4 * qt) * 128
                nsl = slice(roff, _TW)
                ksl = slice(kc * 128, (kc + 1) * 128)
                p_s = pps.tile([128, _TW], f32, tag="ps", name="p_s")
                nc.tensor.matmul(p_s[:, nsl], t_kn[h][:, ksl], t_g[:, nsl],
                                 start=True, stop=True)
                t_a = pc.tile([128, _TW], f32r, tag=f"wk{a_ctr[0] % _NCB}",
                              name=f"a{a_ctr[0] % _NCB}")
                a_ctr[0] += 1
                nc.scalar.activation(t_a[:, nsl], p_s[:, nsl], Act.Exp,
                                     scale=1.0 / float(np.sqrt(_D)))
                if kc >= 4 * qt:  # diagonal chunk: triangular mask
                    dsl = slice(roff, roff + 128)
                    nc.vector.tensor_mul(t_a[:, dsl], t_a[:, dsl], t_tri[:])
                a_tiles[kc] = t_a

            def emit_acc(kc):
                roff = 0 if kc < 4 * qt else (kc - 4 * qt) * 128
                nsl = slice(roff, _TW)
                t_a = a_tiles.pop(kc)
                nc.tensor.matmul(p_den[:, nsl], t_ones_col, t_a[:, nsl],
                                 start=(kc == 0), stop=(kc == nchunk - 1))
                nc.tensor.matmul(p_o[:, nsl], t_v[kc][:, hsl], t_a[:, nsl],
                                 start=(kc == 0), stop=(kc == nchunk - 1))

            for kc in range(nchunk + LOOKAHEAD):
                if kc < nchunk:
                    emit_s(kc)
                if kc >= LOOKAHEAD:
                    emit_acc(kc - LOOKAHEAD)
            # normalize: Z = O_unnorm * (1/den) broadcast
            t_den = psm.tile([1, _TW], f32r, tag="ms", name="t_den", bufs=3)
            nc.scalar.copy(t_den[:], p_den[:])
            p_db = pps.tile([128, _TW], f32, tag="ps", name="p_db")
            nc.tensor.matmul(p_db[:], t_ones_row, t_den[:], start=True, stop=True)
            t_rc2 = psm.tile([128, _TW], f32r, tag="sd", name="t_rc2", bufs=3)
            nc.vector.reciprocal(t_rc2[:], p_db[:])
            zi = h + _HPG * (qt % 2)
            t_z = pc.tile([128, _TW], f32r, tag=f"wv{zi}", name=f"z{zi}")
            nc.vector.tensor_mul(t_z[:], p_o[:], t_rc2[:])
            return t_z

        for qt in range(_NT):
            gs = [q_chain(qt, h) for h in range(_HPG)]
            z_chunks = [attention(qt, h, gs[h]) for h in range(_HPG)]
            # c_proj for this tile: partial out rows [qt*512, qt*512+512)
            for tb in range(4):
                bsl = slice(tb * 128, (tb + 1) * 128)
                t_ob = pob.tile([128, _C], f32, tag="ob")
                for nh in range(2):
                    osl = slice(nh * 384, (nh + 1) * 384)
                    p_c = ppq.tile([128, 384], f32, tag="pq")
                    for c in range(_HPG):
                        nc.tensor.matmul(p_c[:], z_chunks[c][:, bsl],
                                         t_wo[c][:, osl],
                                         start=(c == 0), stop=(c == _HPG - 1))
                    nc.vector.tensor_copy(t_ob[:, osl], p_c[:])
                nc.sync.dma_start(
                    out[qt * _TW + tb * 128: qt * _TW + (tb + 1) * 128, :],
                    t_ob[:])

    nc.compile()
    return nc


def _get_nc():
    if "nc" not in _cached:
        _cached["nc"] = _build_nc()
    return _cached["nc"]


def make_in_maps(x, cos, sin, Wq, Wk, Wv, Wo):
    cosT = np.ascontiguousarray(cos.reshape(_T, _D // 2).T)  # (64, T)
    sinT = np.ascontiguousarray(sin.reshape(_T, _D // 2).T)
    cc = np.concatenate([cosT, cosT], axis=0)                # (128, T)
    ss = np.concatenate([sinT, -sinT], axis=0)
    tri = (np.arange(128)[None, :] >= np.arange(128)[:, None]).astype(np.float32)
    ones128 = np.ones((128, 128), dtype=np.float32)
    permm = np.zeros((128, 128), dtype=np.float32)           # half-swap permutation
    for d in range(64):
        permm[64 + d, d] = 1.0
        permm[d, 64 + d] = 1.0
    in_maps = []
    for core in range(8):
        b, g = divmod(core, 2)
        gsl = slice(g * _HD, (g + 1) * _HD)
        in_maps.append({
            "xT": np.ascontiguousarray(x[b].T),
            "wq": np.ascontiguousarray(Wq[gsl, :].T),
            "wk": np.ascontiguousarray(Wk[gsl, :].T),
            "wv": np.ascontiguousarray(Wv[gsl, :].T),
            "wo": np.ascontiguousarray(Wo[:, gsl].T),
            "cc": cc, "ss": ss, "tri": tri, "ones": ones128, "perm": permm,
        })
    return in_maps


def kernel(x, cos, sin, Wq, Wk, Wv, Wo):
    from concourse.bass_utils import run_bass_kernel_spmd

    x = np.asarray(x, dtype=np.float32)
    cos = np.asarray(cos, dtype=np.float32)
    sin = np.asarray(sin, dtype=np.float32)
    Wq = np.asarray(Wq, dtype=np.float32)
    Wk = np.asarray(Wk, dtype=np.float32)
    Wv = np.asarray(Wv, dtype=np.float32)
    Wo = np.asarray(Wo, dtype=np.float32)

    nc = _get_nc()
    in_maps = make_in_maps(x, cos, sin, Wq, Wk, Wv, Wo)
    res = run_bass_kernel_spmd(nc, in_maps, core_ids=list(range(8)))
    outs = [r_["out"] for r_ in res.results]
    return np.stack([outs[2 * b] + outs[2 * b + 1] for b in range(_B)], axis=0)



# revision 15
# speedup vs baseline: 1.7475x; 1.7475x over previous
"""Trainium2 Bass kernel for CausalSelfAttention (B=4, T=2048, C=768, H=6, D=128)
with RoPE + QK-RMSNorm.

Sharding: 8 cores = batch(4) x head-group(2, 3 heads each). Each core:
  - Q^T/K^T in (D, T) layout, V in (T, D); x/weights stream in as bf16 via
    single rearranged-AP DMAs; RoPE = DVE muls (reading proj PSUM) + a batched
    half-swap DMA; QK-RMSNorm = gpsimd partition_all_reduce + Act ln/exp rsqrt
    (the sqrt(128) factors fold into the attention exp scale)
  - causal attention transposed (S^T: T_k on partitions); softmax denominator
    via ones-matmul PSUM accumulation; z = O/den via DVE recip + gpsimd
    broadcast + DVE mul
  - partial c_proj over its 384 channels; bf16 out partials summed on host
K-tile i+1 / Q-tile qt+1 projections and their rope+norm chains are emitted
inside attention round qt so PE never waits on the elementwise engines.
"""

import numpy as np

_B, _T, _C, _H, _D = 4, 2048, 768, 6, 128
_HPG = 3            # heads per group
_HD = _HPG * _D     # 384
_NT = 4             # T tiles of 512
_TW = 512
_NCB = _C // 128    # 6 c_in chunks
_EPS = 1e-15
_EXPS = float(128.0 / np.sqrt(_D))   # folds the two missing sqrt(128) norms

_cached = {}


def _patch_act_tables():
    """Make Exp and Ln first-match the combined natural_log_exp table so the
    act-table pass emits one load instead of thrashing between the exp-only
    and ln-only tables. Indices stay canonical: we only strip exp/ln from the
    *other* sets in the list the selection pass sees."""
    from concourse import bacc as _bacc
    from concourse import mybir as _mybir
    if getattr(_bacc, "_act_tables_patched", False):
        return
    _orig = _bacc.get_activation_tables
    A = _mybir.ActivationFunctionType

    def patched(arch):
        tabs = _orig(arch)
        names = list(tabs)
        combined = [n for n, fs in tabs.items() if A.Exp in fs and A.Ln in fs]
        if not combined:
            return tabs
        comb = combined[0]
        return {
            n: (fs if n == comb else fs - {A.Exp, A.Ln})
            for n, fs in tabs.items()
        }

    _bacc.get_activation_tables = patched
    _bacc._act_tables_patched = True


def _build_nc():
    from contextlib import ExitStack
    from concourse import bacc, tile, mybir
    from concourse import bass_isa
    _patch_act_tables()

    f32 = mybir.dt.float32
    f32r = mybir.dt.float32r
    bf16 = mybir.dt.bfloat16
    Act = mybir.ActivationFunctionType
    RAdd = bass_isa.ReduceOp.add

    nc = bacc.Bacc("TRN2", target_bir_lowering=False, debug=False)

    xT = nc.dram_tensor("xT", (_C, _T), bf16, kind="ExternalInput").ap()
    wq = nc.dram_tensor("wq", (_C, _HD), bf16, kind="ExternalInput").ap()
    wk = nc.dram_tensor("wk", (_C, _HD), bf16, kind="ExternalInput").ap()
    wv = nc.dram_tensor("wv", (_C, _HD), bf16, kind="ExternalInput").ap()
    wo = nc.dram_tensor("wo", (_HD, _C), f32r, kind="ExternalInput").ap()
    cc = nc.dram_tensor("cc", (128, _T), bf16, kind="ExternalInput").ap()
    ss2 = nc.dram_tensor("ss2", (128, _T), bf16, kind="ExternalInput").ap()
    tri = nc.dram_tensor("tri", (128, 128), f32r, kind="ExternalInput").ap()
    tri2 = nc.dram_tensor("tri2", (128, 256), f32r, kind="ExternalInput").ap()
    out = nc.dram_tensor("out", (_T, _C), bf16, kind="ExternalOutput").ap()

    with tile.TileContext(nc) as tc, ExitStack() as ctx, \
            nc.allow_low_precision(reason="f32r tiles carry fp32 bits; PE rounds at ingest"):
        # --- SBUF pools ---
        pc = ctx.enter_context(tc.tile_pool(name="pc", bufs=1))      # persistent
        pb = ctx.enter_context(tc.tile_pool(name="pb", bufs=2))      # t_b3 (x*ss2)
        pw = ctx.enter_context(tc.tile_pool(name="pw", bufs=2))      # t_bsw3
        psq = ctx.enter_context(tc.tile_pool(name="psq", bufs=2))    # squares
        pms = ctx.enter_context(tc.tile_pool(name="pms", bufs=2))    # allred out
        pq = ctx.enter_context(tc.tile_pool(name="pq", bufs=2))      # Q tiles
        pa = ctx.enter_context(tc.tile_pool(name="pa", bufs=3))      # A chunks
        pz = ctx.enter_context(tc.tile_pool(name="pz", bufs=2))      # z3 tiles
        prc = ctx.enter_context(tc.tile_pool(name="prc", bufs=2))    # recip den
        pdb = ctx.enter_context(tc.tile_pool(name="pdb", bufs=2))    # den bcast
        pob = ctx.enter_context(tc.tile_pool(name="pob", bufs=3))    # out staging
        # --- PSUM pools (8 banks) ---
        pp = ctx.enter_context(tc.tile_pool(name="pp", bufs=3, space="PSUM"))
        pps = ctx.enter_context(tc.tile_pool(name="pps", bufs=2, space="PSUM"))
        ppo = ctx.enter_context(tc.tile_pool(name="ppo", bufs=2, space="PSUM"))
        ppd = ctx.enter_context(tc.tile_pool(name="ppd", bufs=1, space="PSUM"))

        # --- persistent tiles ---
        t_x = pc.tile([128, _NCB * _T], bf16, tag="x")     # chunk c at [c*T,)
        t_wk3 = pc.tile([128, _NCB * _HD], bf16, tag="wk3")
        t_wv3 = pc.tile([128, _NCB * _HD], bf16, tag="wv3")
        t_wq3 = pc.tile([128, _NCB * _HD], bf16, tag="wq3")
        t_wo3 = pc.tile([128, _HPG * _C], f32r, tag="wo3")
        t_cc = pc.tile([128, _T], bf16, tag="cc")
        t_ss = pc.tile([128, _T], bf16, tag="ss2")
        t_tri = pc.tile([128, 128], f32r, tag="tri")
        t_tri2 = pc.tile([128, 256], f32r, tag="tri2")
        t_eps = pc.tile([128, 1], f32, tag="eps")
        nc.gpsimd.memset(t_eps[:], float(128.0 * _EPS))
        # K^T (rope + scaled norm): head h at cols [h*T, (h+1)*T)
        t_kn3 = pc.tile([128, _HPG * _T], f32r, tag="kn3")
        t_v = [pc.tile([128, _HD], f32r, tag=f"v{tb}", name=f"v{tb}")
               for tb in range(_T // 128)]

        def xs(c, lo, hi):
            return t_x[:, c * _T + lo: c * _T + hi]

        def wslice(tw, c, hsl):
            return tw[:, c * _HD + hsl.start: c * _HD + hsl.stop]

        # ---- batched loads (one DMA per tensor / x-tile) ----
        def load_xt_tile(i, chunks=None):
            isl = slice(i * _TW, (i + 1) * _TW)
            lo, hi = (0, _NCB) if chunks is None else chunks
            o3 = t_x[:].rearrange("p (c t) -> p c t", c=_NCB)[:, lo:hi, isl]
            i3 = xT[:, isl].rearrange("(c p) f -> p c f", c=_NCB)[:, lo:hi, :]
            nc.sync.dma_start(o3, i3)

        def load_w(dst, src, nblk):
            o3 = dst[:].rearrange("p (c f) -> p c f", c=nblk)
            nc.sync.dma_start(o3, src.rearrange("(c p) f -> p c f", c=nblk))

        # ---------- rope + norm chains ----------
        def rope_mul(y_ap, b_ap, p_ap, col0):
            csl = slice(col0, col0 + _TW)
            nc.vector.tensor_mul(b_ap, p_ap, t_ss[:, csl])    # x*ss2
            nc.vector.tensor_mul(y_ap, p_ap, t_cc[:, csl])    # x*cc

        def chain_swap(t_b3):
            """Batched half-swap DMA of the 3 heads' x*ss2."""
            t_bsw = pw.tile([128, _HPG * _TW], f32r, tag="bsw", name="t_bsw")
            nc.sync.dma_start(t_bsw[0:64, :], t_b3[64:128, :])
            nc.sync.dma_start(t_bsw[64:128, :], t_b3[0:64, :])
            return t_bsw

        def chain_part1(y3_ap, t_bsw):
            """Batched DVE add + square, Pool all-reduce. Emit one attention
            head after chain_swap so the DVE queue never waits on the DMA."""
            b3 = t_bsw[:].rearrange("p (h t) -> p h t", h=_HPG)
            nc.vector.tensor_add(y3_ap, y3_ap, b3)
            t_sq = psq.tile([128, _HPG * _TW], f32r, tag="sq", name="t_sq")
            s3 = t_sq[:].rearrange("p (h t) -> p h t", h=_HPG)
            nc.vector.tensor_mul(s3, y3_ap, y3_ap)
            t_ms = pms.tile([128, _HPG * _TW], f32r, tag="ms", name="t_ms")
            nc.gpsimd.partition_all_reduce(t_ms[:], t_sq[:], channels=128,
                                           reduce_op=RAdd)
            return t_ms

        def chain_part2(y_aps, t_ms, heads=range(_HPG)):
            """Per-head ln+exp rsqrt (Act) + scale mul (DVE) — emitted late so
            the Act queue never blocks attention exps."""
            for h in heads:
                hsl = slice(h * _TW, (h + 1) * _TW)
                nc.scalar.activation(t_ms[:, hsl], t_ms[:, hsl], Act.Ln,
                                     bias=t_eps[:], scale=1.0)
                nc.scalar.activation(t_ms[:, hsl], t_ms[:, hsl], Act.Exp,
                                     scale=-0.5)
                nc.vector.tensor_mul(y_aps[h], y_aps[h], t_ms[:, hsl])

        def kq_projs(i, wtile, is_k):
            """3 head projections for K tile i (or Q tile i) + rope muls.
            Returns (y_aps, y3_view, t_b3) for chain_part1/2."""
            isl = slice(i * _TW, (i + 1) * _TW)
            t_b3 = pb.tile([128, _HPG * _TW], f32r, tag="b3", name="t_b3")
            y_aps = []
            if is_k:
                y3 = t_kn3[:].rearrange("p (h t) -> p h t",
                                        h=_HPG)[:, :, isl]
            else:
                t_g3 = pq.tile([128, _HPG * _TW], f32r, tag="g", name="t_g3")
                y3 = t_g3[:].rearrange("p (h t) -> p h t", h=_HPG)
            for h in range(_HPG):
                hsl = slice(h * 128, (h + 1) * 128)
                p_ = pp.tile([128, _TW], f32, tag="pp", name="p_")
                for c in range(_NCB):
                    nc.tensor.matmul(p_[:], wslice(wtile, c, hsl),
                                     xs(c, isl.start, isl.stop),
                                     start=(c == 0), stop=(c == _NCB - 1))
                if is_k:
                    y = t_kn3[:, h * _T + isl.start: h * _T + isl.stop]
                else:
                    y = t_g3[:, h * _TW:(h + 1) * _TW]
                rope_mul(y, t_b3[:, h * _TW:(h + 1) * _TW], p_[:], isl.start)
                y_aps.append(y)
            return y_aps, y3, t_b3

        def vprojs(tbs, copies_on_act):
            for tb in tbs:
                p_v = pp.tile([128, _HD], f32, tag="pp", name="p_v",
                              padded_shape=[128, _TW])
                for c in range(_NCB):
                    nc.tensor.matmul(p_v[:], xs(c, tb * 128, (tb + 1) * 128),
                                     t_wv3[:, c * _HD:(c + 1) * _HD],
                                     start=(c == 0), stop=(c == _NCB - 1))
                if copies_on_act:
                    nc.scalar.copy(t_v[tb][:], p_v[:])
                else:
                    nc.vector.tensor_copy(t_v[tb][:], p_v[:])

        # ---------------- attention ----------------
        def attention(qt, h, t_g, z3):
            nchunk = 4 * qt + 4
            LOOK = 3
            hsl = slice(h * 128, (h + 1) * 128)
            p_den = ppd.tile([1, _TW], f32, tag="pd", name="p_den")
            p_o = ppo.tile([128, _TW], f32, tag="po", name="p_o")
            a_tiles = {}

            def nsl_of(kc):
                d = kc - 4 * qt
                if d < 0:
                    return slice(0, _TW)
                if d >= 3:
                    return slice(256, _TW)
                return slice(d * 128, _TW)

            def emit_s(kc):
                nsl = nsl_of(kc)
                p_s = pps.tile([128, _TW], f32, tag="ps", name="p_s")
                nc.tensor.matmul(p_s[:, nsl],
                                 t_kn3[:, h * _T + kc * 128:
                                       h * _T + (kc + 1) * 128],
                                 t_g[:, nsl], start=True, stop=True)
                t_a = pa.tile([128, _TW], f32r, tag="a", name="t_a")
                nc.scalar.activation(t_a[:, nsl], p_s[:, nsl], Act.Exp,
                                     scale=_EXPS)
                d = kc - 4 * qt
                if d >= 3:
                    nc.vector.tensor_mul(t_a[:, 256:_TW], t_a[:, 256:_TW],
                                         t_tri2[:])
                elif d >= 0:
                    dsl = slice(d * 128, (d + 1) * 128)
                    nc.vector.tensor_mul(t_a[:, dsl], t_a[:, dsl], t_tri[:])
                a_tiles[kc] = t_a

            def emit_acc(kc):
                nsl = nsl_of(kc)
                t_a = a_tiles.pop(kc)
                nc.tensor.matmul(p_den[:, nsl], t_tri[:, 127:128], t_a[:, nsl],
                                 start=(kc == 0), stop=(kc == nchunk - 1))
                nc.tensor.matmul(p_o[:, nsl], t_v[kc][:, hsl], t_a[:, nsl],
                                 start=(kc == 0), stop=(kc == nchunk - 1))

            for kc in range(nchunk + LOOK):
                if kc < nchunk:
                    emit_s(kc)
                if kc >= LOOK:
                    emit_acc(kc - LOOK)
            t_rc = prc.tile([1, _TW], f32r, tag="rc", name="t_rc")
            nc.vector.reciprocal(t_rc[:], p_den[:])
            t_db = pdb.tile([128, _TW], f32r, tag="db", name="t_db")
            nc.gpsimd.partition_broadcast(t_db[:], t_rc[:])
            nc.vector.tensor_mul(z3[:, h * _TW:(h + 1) * _TW], p_o[:], t_db[:])

        def cproj(qt, z3):
            rsl0 = qt * _TW
            for tb in range(4):
                t_ob = pob.tile([128, _C], bf16, tag="ob", name="t_ob")
                for nh in range(2):
                    osl = slice(nh * 384, (nh + 1) * 384)
                    p_c = pps.tile([128, 384], f32, tag="ps", name="p_c",
                                   padded_shape=[128, _TW])
                    for c in range(_HPG):
                        nc.tensor.matmul(
                            p_c[:], z3[:, c * _TW + tb * 128:
                                       c * _TW + (tb + 1) * 128],
                            t_wo3[:, c * _C + osl.start: c * _C + osl.stop],
                            start=(c == 0), stop=(c == _HPG - 1))
                    if nh == 0:
                        nc.vector.tensor_copy(t_ob[:, osl], p_c[:])
                    else:
                        nc.scalar.copy(t_ob[:, osl], p_c[:])
                    if qt == _NT - 1:
                        nc.sync.dma_start(
                            out[rsl0 + tb * 128: rsl0 + (tb + 1) * 128, osl],
                            t_ob[:, osl])
                if qt < _NT - 1:
                    nc.sync.dma_start(
                        out[rsl0 + tb * 128: rsl0 + (tb + 1) * 128, :],
                        t_ob[:])

        # ================= emission =================
        load_w(t_wk3, wk, _NCB)
        load_xt_tile(0, (0, 3))
        load_xt_tile(0, (3, 6))
        nc.sync.dma_start(t_cc[:], cc[:])
        nc.sync.dma_start(t_ss[:], ss2[:])
        load_w(t_wq3, wq, _NCB)
        load_w(t_wv3, wv, _NCB)
        load_xt_tile(1)
        nc.sync.dma_start(t_tri[:], tri[:])
        nc.sync.dma_start(t_tri2[:], tri2[:])
        load_xt_tile(2)
        load_xt_tile(3)
        load_w(t_wo3, wo, _HPG)

        # prologue: K tile 0, Q tile 0, V blocks 0..7
        yk, yk3, bk = kq_projs(0, t_wk3, True)
        bswk = chain_swap(bk[:])
        yq, yq3, bq = kq_projs(0, t_wq3, False)
        bswq = chain_swap(bq[:])
        msk = chain_part1(yk3, bswk)
        msq = chain_part1(yq3, bswq)
        vprojs(range(0, 8), True)
        chain_part2(yk, msk)
        chain_part2(yq, msq)
        gs = yq

        for qt in range(_NT):
            z3 = pz.tile([128, _HPG * _TW], f32r, tag="z3", name="z3")
            yk = yq2 = None
            for h in range(_HPG):
                attention(qt, h, gs[h], z3)
                # interleave next-tile projections after each head; the
                # DVE add/sq of each chain is deferred one head so it never
                # head-of-line blocks on its swap DMA
                if qt < _NT - 1:
                    if h == 0:
                        yk, yk3, bk = kq_projs(qt + 1, t_wk3, True)
                        bswk = chain_swap(bk[:])
                    elif h == 1:
                        yq2, yq23, bq2 = kq_projs(qt + 1, t_wq3, False)
                        bswq = chain_swap(bq2[:])
                        msk = chain_part1(yk3, bswk)
                    else:
                        if qt < _NT - 2:
                            vprojs(range(4 * qt + 8, 4 * qt + 12), False)
                        msq = chain_part1(yq23, bswq)
            if qt < _NT - 1:
                # K rsqrt+scale before cproj: att(qt+1,h0) reads the new
                # kn tile mid-head, so finish K as early as possible
                chain_part2(yk, msk)
            cproj(qt, z3)
            if qt < _NT - 1:
                chain_part2(yq2, msq)
            gs = yq2

    nc.compile()
    return nc


def _get_nc():
    if "nc" not in _cached:
        _cached["nc"] = _build_nc()
    return _cached["nc"]


def make_in_maps(x, cos, sin, Wq, Wk, Wv, Wo):
    import ml_dtypes
    bf = ml_dtypes.bfloat16
    cosT = np.ascontiguousarray(cos.reshape(_T, _D // 2).T)  # (64, T)
    sinT = np.ascontiguousarray(sin.reshape(_T, _D // 2).T)
    ccm = np.concatenate([cosT, cosT], axis=0).astype(bf)
    ss2m = np.concatenate([-sinT, sinT], axis=0).astype(bf)  # swap-signed sin
    trim = (np.arange(128)[None, :] >= np.arange(128)[:, None]).astype(np.float32)
    tri2m = np.concatenate([np.zeros((128, 128), np.float32), trim], axis=1)
    in_maps = []
    for core in range(8):
        b, g = divmod(core, 2)
        gsl = slice(g * _HD, (g + 1) * _HD)
        in_maps.append({
            "xT": np.ascontiguousarray(x[b].T).astype(bf),
            "wq": np.ascontiguousarray(Wq[gsl, :].T).astype(bf),
            "wk": np.ascontiguousarray(Wk[gsl, :].T).astype(bf),
            "wv": np.ascontiguousarray(Wv[gsl, :].T).astype(bf),
            "wo": np.ascontiguousarray(Wo[:, gsl].T),
            "cc": ccm, "ss2": ss2m, "tri": trim, "tri2": tri2m,
        })
    return in_maps


def kernel(x, cos, sin, Wq, Wk, Wv, Wo):
    from concourse.bass_utils import run_bass_kernel_spmd

    x = np.asarray(x, dtype=np.float32)
    cos = np.asarray(cos, dtype=np.float32)
    sin = np.asarray(sin, dtype=np.float32)
    Wq = np.asarray(Wq, dtype=np.float32)
    Wk = np.asarray(Wk, dtype=np.float32)
    Wv = np.asarray(Wv, dtype=np.float32)
    Wo = np.asarray(Wo, dtype=np.float32)

    nc = _get_nc()
    in_maps = make_in_maps(x, cos, sin, Wq, Wk, Wv, Wo)
    res = run_bass_kernel_spmd(nc, in_maps, core_ids=list(range(8)))
    outs = [np.asarray(r_["out"]).astype(np.float32) for r_ in res.results]
    return np.stack([outs[2 * b] + outs[2 * b + 1] for b in range(_B)], axis=0)


# revision 35
# speedup vs baseline: 1.7510x; 1.0020x over previous
"""Trainium2 Bass kernel for CausalSelfAttention (B=4, T=2048, C=768, H=6, D=128)
with RoPE + QK-RMSNorm.

Sharding: 8 cores = batch(4) x head-group(2, 3 heads each). Each core:
  - Q^T/K^T in (D, T) layout, V in (T, D); x/weights stream in as bf16 via
    single rearranged-AP DMAs; RoPE = DVE muls (reading proj PSUM) + a batched
    half-swap DMA; QK-RMSNorm = gpsimd partition_all_reduce + Act ln/exp rsqrt
    (the sqrt(128) factors fold into the attention exp scale)
  - causal attention transposed (S^T: T_k on partitions); softmax denominator
    via ones-matmul PSUM accumulation; z = O/den via DVE recip + gpsimd
    broadcast + DVE mul
  - partial c_proj over its 384 channels; bf16 out partials summed on host
K-tile i+1 / Q-tile qt+1 projections and their rope+norm chains are emitted
inside attention round qt so PE never waits on the elementwise engines.
"""

import numpy as np

_B, _T, _C, _H, _D = 4, 2048, 768, 6, 128
_HPG = 3            # heads per group
_HD = _HPG * _D     # 384
_NT = 4             # T tiles of 512
_TW = 512
_NCB = _C // 128    # 6 c_in chunks
_EPS = 1e-15
_EXPS = float(128.0 / np.sqrt(_D))   # folds the two missing sqrt(128) norms

_cached = {}


def _patch_act_tables():
    """Make Exp and Ln first-match the combined natural_log_exp table so the
    act-table pass emits one load instead of thrashing between the exp-only
    and ln-only tables. Indices stay canonical: we only strip exp/ln from the
    *other* sets in the list the selection pass sees."""
    from concourse import bacc as _bacc
    from concourse import mybir as _mybir
    if getattr(_bacc, "_act_tables_patched", False):
        return
    _orig = _bacc.get_activation_tables
    A = _mybir.ActivationFunctionType

    def patched(arch):
        tabs = _orig(arch)
        names = list(tabs)
        combined = [n for n, fs in tabs.items() if A.Exp in fs and A.Ln in fs]
        if not combined:
            return tabs
        comb = combined[0]
        return {
            n: (fs if n == comb else fs - {A.Exp, A.Ln})
            for n, fs in tabs.items()
        }

    _bacc.get_activation_tables = patched
    _bacc._act_tables_patched = True


def _build_nc():
    from contextlib import ExitStack
    from concourse import bacc, tile, mybir
    from concourse import bass_isa
    _patch_act_tables()

    f32 = mybir.dt.float32
    f32r = mybir.dt.float32r
    bf16 = mybir.dt.bfloat16
    Act = mybir.ActivationFunctionType
    RAdd = bass_isa.ReduceOp.add

    nc = bacc.Bacc("TRN2", target_bir_lowering=False, debug=False)

    xT = nc.dram_tensor("xT", (_C, _T), bf16, kind="ExternalInput").ap()
    wq = nc.dram_tensor("wq", (_C, _HD), bf16, kind="ExternalInput").ap()
    wk = nc.dram_tensor("wk", (_C, _HD), bf16, kind="ExternalInput").ap()
    wv = nc.dram_tensor("wv", (_C, _HD), bf16, kind="ExternalInput").ap()
    wo = nc.dram_tensor("wo", (_HD, _C), bf16, kind="ExternalInput").ap()
    cc = nc.dram_tensor("cc", (128, _T), bf16, kind="ExternalInput").ap()
    ss2 = nc.dram_tensor("ss2", (128, _T), bf16, kind="ExternalInput").ap()
    tri = nc.dram_tensor("tri", (128, 128), bf16, kind="ExternalInput").ap()
    tri2 = nc.dram_tensor("tri2", (128, 256), bf16, kind="ExternalInput").ap()
    out = nc.dram_tensor("out", (_T, _C), bf16, kind="ExternalOutput").ap()

    with tile.TileContext(nc) as tc, ExitStack() as ctx, \
            nc.allow_low_precision(reason="f32r tiles carry fp32 bits; PE rounds at ingest"):
        # --- SBUF pools ---
        pc = ctx.enter_context(tc.tile_pool(name="pc", bufs=1))      # persistent
        pb = ctx.enter_context(tc.tile_pool(name="pb", bufs=2))      # t_b3 (x*ss2)
        pw = ctx.enter_context(tc.tile_pool(name="pw", bufs=2))      # t_bsw3
        psq = ctx.enter_context(tc.tile_pool(name="psq", bufs=2))    # squares
        pms = ctx.enter_context(tc.tile_pool(name="pms", bufs=2))    # allred out
        pq = ctx.enter_context(tc.tile_pool(name="pq", bufs=2))      # Q tiles
        pa = ctx.enter_context(tc.tile_pool(name="pa", bufs=6))      # A chunks
        pz = ctx.enter_context(tc.tile_pool(name="pz", bufs=2))      # z3 tiles
        prc = ctx.enter_context(tc.tile_pool(name="prc", bufs=2))    # recip den
        pdb = ctx.enter_context(tc.tile_pool(name="pdb", bufs=2))    # den bcast
        pob = ctx.enter_context(tc.tile_pool(name="pob", bufs=3))    # out staging
        # --- PSUM pools (8 banks) ---
        pp = ctx.enter_context(tc.tile_pool(name="pp", bufs=3, space="PSUM"))
        pps = ctx.enter_context(tc.tile_pool(name="pps", bufs=2, space="PSUM"))
        ppo = ctx.enter_context(tc.tile_pool(name="ppo", bufs=2, space="PSUM"))
        ppd = ctx.enter_context(tc.tile_pool(name="ppd", bufs=1, space="PSUM"))

        # --- persistent tiles ---
        t_x = pc.tile([128, _NCB * _T], bf16, tag="x")     # chunk c at [c*T,)
        t_wk3 = pc.tile([128, _NCB * _HD], bf16, tag="wk3")
        t_wv3 = pc.tile([128, _NCB * _HD], bf16, tag="wv3")
        t_wq3 = pc.tile([128, _NCB * _HD], bf16, tag="wq3")
        t_wo3 = pc.tile([128, _HPG * _C], bf16, tag="wo3")
        t_cc = pc.tile([128, _T], bf16, tag="cc")
        t_ss = pc.tile([128, _T], bf16, tag="ss2")
        t_tri = pc.tile([128, 128], bf16, tag="tri")
        t_tri2 = pc.tile([128, 256], bf16, tag="tri2")
        t_eps = pc.tile([128, 1], f32, tag="eps")
        nc.gpsimd.memset(t_eps[:], float(128.0 * _EPS))
        # K^T (rope + scaled norm): head h at cols [h*T, (h+1)*T)
        t_kn3 = pc.tile([128, _HPG * _T], f32r, tag="kn3")
        t_v = [pc.tile([128, _HD], bf16, tag=f"v{tb}", name=f"v{tb}")
               for tb in range(_T // 128)]
        p_den2 = ppd.tile([64, _TW], f32, tag="pd", name="p_den2")

        def xs(c, lo, hi):
            return t_x[:, c * _T + lo: c * _T + hi]

        def wslice(tw, c, hsl):
            return tw[:, c * _HD + hsl.start: c * _HD + hsl.stop]

        # ---- batched loads (one DMA per tensor / x-tile) ----
        def load_xt_tile(i, chunks=None):
            isl = slice(i * _TW, (i + 1) * _TW)
            lo, hi = (0, _NCB) if chunks is None else chunks
            o3 = t_x[:].rearrange("p (c t) -> p c t", c=_NCB)[:, lo:hi, isl]
            i3 = xT[:, isl].rearrange("(c p) f -> p c f", c=_NCB)[:, lo:hi, :]
            nc.sync.dma_start(o3, i3)

        def load_w(dst, src, nblk):
            o3 = dst[:].rearrange("p (c f) -> p c f", c=nblk)
            nc.sync.dma_start(o3, src.rearrange("(c p) f -> p c f", c=nblk))

        # ---------- rope + norm chains ----------
        def rope_mul(y_ap, b_ap, p_ap, col0):
            csl = slice(col0, col0 + _TW)
            nc.vector.tensor_mul(b_ap, p_ap, t_ss[:, csl])    # x*ss2
            nc.vector.tensor_mul(y_ap, p_ap, t_cc[:, csl])    # x*cc

        def chain_swap(t_b3):
            """Batched half-swap DMA of the 3 heads' x*ss2."""
            t_bsw = pw.tile([128, _HPG * _TW], f32r, tag="bsw", name="t_bsw")
            nc.sync.dma_start(t_bsw[0:64, :], t_b3[64:128, :])
            nc.sync.dma_start(t_bsw[64:128, :], t_b3[0:64, :])
            return t_bsw

        def chain_part1(y3_ap, t_bsw, on_pool=False):
            """Batched add + square (DVE, or Pool when DVE is the round's
            bottleneck), Pool all-reduce. Emitted one attention head after
            chain_swap so the queues never wait on the DMA."""
            eng = nc.gpsimd if on_pool else nc.vector
            b3 = t_bsw[:].rearrange("p (h t) -> p h t", h=_HPG)
            eng.tensor_add(y3_ap, y3_ap, b3)
            t_sq = psq.tile([128, _HPG * _TW], f32r, tag="sq", name="t_sq")
            s3 = t_sq[:].rearrange("p (h t) -> p h t", h=_HPG)
            eng.tensor_mul(s3, y3_ap, y3_ap)
            t_ms = pms.tile([128, _HPG * _TW], f32r, tag="ms", name="t_ms")
            nc.gpsimd.partition_all_reduce(t_ms[:], t_sq[:], channels=128,
                                           reduce_op=RAdd)
            return t_ms

        def chain_part2(y_aps, t_ms):
            """Per-head ln+exp rsqrt (Act) + scale mul (DVE) — emitted late so
            the Act queue never blocks attention exps."""
            for h in range(_HPG):
                hsl = slice(h * _TW, (h + 1) * _TW)
                nc.scalar.activation(t_ms[:, hsl], t_ms[:, hsl], Act.Ln,
                                     bias=t_eps[:], scale=1.0)
                nc.scalar.activation(t_ms[:, hsl], t_ms[:, hsl], Act.Exp,
                                     scale=-0.5)
                nc.vector.tensor_mul(y_aps[h], y_aps[h], t_ms[:, hsl])

        def chain_solo(y, b_ap):
            """Low-latency per-head chain: swap DMAs + DVE add/sq + Pool
            all-reduce + Act ln/exp + DVE mul. Prologue only."""
            t_bsw = pw.tile([128, _TW], f32r, tag="bsw1", name="t_bsw1",
                            bufs=2)
            nc.sync.dma_start(t_bsw[0:64, :], b_ap[64:128, :])
            nc.sync.dma_start(t_bsw[64:128, :], b_ap[0:64, :])
            nc.vector.tensor_add(y, y, t_bsw[:])
            t_sq = psq.tile([128, _TW], f32r, tag="sq1", name="t_sq1", bufs=2)
            nc.vector.tensor_mul(t_sq[:], y, y)
            t_ms = pms.tile([128, _TW], f32r, tag="ms1", name="t_ms1", bufs=2)
            nc.gpsimd.partition_all_reduce(t_ms[:], t_sq[:], channels=128,
                                           reduce_op=RAdd)
            nc.scalar.activation(t_ms[:], t_ms[:], Act.Ln,
                                 bias=t_eps[:], scale=1.0)
            nc.scalar.activation(t_ms[:], t_ms[:], Act.Exp, scale=-0.5)
            nc.vector.tensor_mul(y, y, t_ms[:])

        def kq_projs(i, wtile, is_k):
            """3 head projections for K tile i (or Q tile i) + rope muls.
            Returns (y_aps, y3_view, t_b3) for chain_part1/2."""
            isl = slice(i * _TW, (i + 1) * _TW)
            t_b3 = pb.tile([128, _HPG * _TW], f32r, tag="b3", name="t_b3")
            y_aps = []
            if is_k:
                y3 = t_kn3[:].rearrange("p (h t) -> p h t",
                                        h=_HPG)[:, :, isl]
            else:
                t_g3 = pq.tile([128, _HPG * _TW], f32r, tag="g", name="t_g3")
                y3 = t_g3[:].rearrange("p (h t) -> p h t", h=_HPG)
            for h in range(_HPG):
                hsl = slice(h * 128, (h + 1) * 128)
                p_ = pp.tile([128, _TW], f32, tag="pp", name="p_")
                for c in range(_NCB):
                    nc.tensor.matmul(p_[:], wslice(wtile, c, hsl),
                                     xs(c, isl.start, isl.stop),
                                     start=(c == 0), stop=(c == _NCB - 1))
                if is_k:
                    y = t_kn3[:, h * _T + isl.start: h * _T + isl.stop]
                else:
                    y = t_g3[:, h * _TW:(h + 1) * _TW]
                rope_mul(y, t_b3[:, h * _TW:(h + 1) * _TW], p_[:], isl.start)
                y_aps.append(y)
            return y_aps, y3, t_b3

        def vprojs(tbs, copies_on_act):
            for tb in tbs:
                p_v = pp.tile([128, _HD], f32, tag="pp", name="p_v",
                              padded_shape=[128, _TW])
                for c in range(_NCB):
                    nc.tensor.matmul(p_v[:], xs(c, tb * 128, (tb + 1) * 128),
                                     t_wv3[:, c * _HD:(c + 1) * _HD],
                                     start=(c == 0), stop=(c == _NCB - 1))
                if copies_on_act:
                    nc.scalar.copy(t_v[tb][:], p_v[:])
                else:
                    nc.vector.tensor_copy(t_v[tb][:], p_v[:])

        # ---------------- attention ----------------
        def attention(qt, heads, t_gs, z3, p_den_rows):
            """Chunk-interleaved attention for 1 or 2 heads of T_q tile qt.
            p_den_rows: per-head base row (0/32) in the shared den psum."""
            nchunk = 4 * qt + 4
            LOOK = 3 if len(heads) == 1 else 2
            st = {}
            for h in heads:
                st[h] = dict(
                    p_o=ppo.tile([128, _TW], f32, tag="po", name="p_o"),
                    a={})

            def nsl_of(kc):
                d = kc - 4 * qt
                if d < 0:
                    return slice(0, _TW)
                if d >= 3:
                    return slice(256, _TW)
                return slice(d * 128, _TW)

            def emit_s(h, kc):
                nsl = nsl_of(kc)
                p_s = pps.tile([128, _TW], f32, tag="ps", name="p_s")
                nc.tensor.matmul(p_s[:, nsl],
                                 t_kn3[:, h * _T + kc * 128:
                                       h * _T + (kc + 1) * 128],
                                 t_gs[h][:, nsl], start=True, stop=True)
                t_a = pa.tile([128, _TW], bf16, tag="a", name="t_a")
                nc.scalar.activation(t_a[:, nsl], p_s[:, nsl], Act.Exp,
                                     scale=_EXPS)
                d = kc - 4 * qt
                if d >= 3:
                    nc.vector.tensor_mul(t_a[:, 256:_TW], t_a[:, 256:_TW],
                                         t_tri2[:])
                elif d >= 0:
                    dsl = slice(d * 128, (d + 1) * 128)
                    nc.vector.tensor_mul(t_a[:, dsl], t_a[:, dsl], t_tri[:])
                st[h]["a"][kc] = t_a

            def emit_acc(h, kc):
                nsl = nsl_of(kc)
                hsl = slice(h * 128, (h + 1) * 128)
                t_a = st[h]["a"].pop(kc)
                r = p_den_rows[h]
                nc.tensor.matmul(p_den2[r:r + 1, nsl], t_tri[:, 127:128],
                                 t_a[:, nsl],
                                 start=(kc == 0), stop=(kc == nchunk - 1))
                nc.tensor.matmul(st[h]["p_o"][:, nsl], t_v[kc][:, hsl],
                                 t_a[:, nsl],
                                 start=(kc == 0), stop=(kc == nchunk - 1))

            for kc in range(nchunk + LOOK):
                for h in heads:
                    if kc < nchunk:
                        emit_s(h, kc)
                for h in heads:
                    if kc >= LOOK:
                        emit_acc(h, kc - LOOK)
            for h in heads:
                r = p_den_rows[h]
                t_rc = prc.tile([1, _TW], f32r, tag="rc", name="t_rc")
                nc.vector.reciprocal(t_rc[:], p_den2[r:r + 1, :])
                t_db = pdb.tile([128, _TW], f32r, tag="db", name="t_db")
                nc.gpsimd.partition_broadcast(t_db[:], t_rc[:])
                nc.vector.tensor_mul(z3[:, h * _TW:(h + 1) * _TW],
                                     st[h]["p_o"][:], t_db[:])

        def cproj(qt, z3):
            rsl0 = qt * _TW
            for tb in range(4):
                t_ob = pob.tile([128, _C], bf16, tag="ob", name="t_ob")
                for nh in range(2):
                    osl = slice(nh * 384, (nh + 1) * 384)
                    p_c = pps.tile([128, 384], f32, tag="ps", name="p_c",
                                   padded_shape=[128, _TW])
                    for c in range(_HPG):
                        nc.tensor.matmul(
                            p_c[:], z3[:, c * _TW + tb * 128:
                                       c * _TW + (tb + 1) * 128],
                            t_wo3[:, c * _C + osl.start: c * _C + osl.stop],
                            start=(c == 0), stop=(c == _HPG - 1))
                    nc.vector.tensor_copy(t_ob[:, osl], p_c[:])
                    if qt == _NT - 1:
                        nc.sync.dma_start(
                            out[rsl0 + tb * 128: rsl0 + (tb + 1) * 128, osl],
                            t_ob[:, osl])
                if qt < _NT - 1:
                    nc.sync.dma_start(
                        out[rsl0 + tb * 128: rsl0 + (tb + 1) * 128, :],
                        t_ob[:])

        # ================= emission =================
        load_w(t_wk3, wk, _NCB)
        load_xt_tile(0, (0, 3))
        load_xt_tile(0, (3, 6))
        nc.sync.dma_start(t_cc[:], cc[:])
        nc.sync.dma_start(t_ss[:], ss2[:])
        load_w(t_wq3, wq, _NCB)
        load_w(t_wv3, wv, _NCB)
        load_xt_tile(1)
        nc.sync.dma_start(t_tri[:], tri[:])
        nc.sync.dma_start(t_tri2[:], tri2[:])
        load_xt_tile(2)
        load_xt_tile(3)
        load_w(t_wo3, wo, _HPG)

        # prologue: K tile 0, Q tile 0, V blocks 0..7
        yk, yk3, bk = kq_projs(0, t_wk3, True)
        bswk = chain_swap(bk[:])
        yq, yq3, bq = kq_projs(0, t_wq3, False)
        bswq = chain_swap(bq[:])
        msk = chain_part1(yk3, bswk)
        msq = chain_part1(yq3, bswq)
        vprojs(range(0, 8), True)
        chain_part2(yk, msk)
        chain_part2(yq, msq)
        gs = yq

        z3_prev = None
        for qt in range(_NT):
            z3 = pz.tile([128, _HPG * _TW], bf16, tag="z3", name="z3")
            attention(qt, (0, 1), {0: gs[0], 1: gs[1]}, z3, {0: 0, 1: 32})
            if qt < _NT - 1:
                yk, yk3, bk = kq_projs(qt + 1, t_wk3, True)
                bswk = chain_swap(bk[:])
                yq2, yq23, bq2 = kq_projs(qt + 1, t_wq3, False)
                bswq = chain_swap(bq2[:])
                if qt < _NT - 2:
                    vprojs(range(4 * qt + 8, 4 * qt + 12), True)
            if z3_prev is not None:
                cproj(qt - 1, z3_prev)
            attention(qt, (2,), {2: gs[2]}, z3, {2: 0})
            if qt < _NT - 1:
                msk = chain_part1(yk3, bswk)
                msq = chain_part1(yq23, bswq)
                chain_part2(yk, msk)
                chain_part2(yq2, msq)
            z3_prev = z3
            gs = yq2 if qt < _NT - 1 else None
        cproj(_NT - 1, z3_prev)

    nc.compile()
    return nc


def _get_nc():
    if "nc" not in _cached:
        _cached["nc"] = _build_nc()
    return _cached["nc"]


def make_in_maps(x, cos, sin, Wq, Wk, Wv, Wo):
    import ml_dtypes
    bf = ml_dtypes.bfloat16
    cosT = np.ascontiguousarray(cos.reshape(_T, _D // 2).T)  # (64, T)
    sinT = np.ascontiguousarray(sin.reshape(_T, _D // 2).T)
    ccm = np.concatenate([cosT, cosT], axis=0).astype(bf)
    ss2m = np.concatenate([-sinT, sinT], axis=0).astype(bf)  # swap-signed sin
    trim = (np.arange(128)[None, :] >= np.arange(128)[:, None]).astype(bf)
    tri2m = np.concatenate([np.zeros((128, 128), bf), trim], axis=1)
    in_maps = []
    for core in range(8):
        b, g = divmod(core, 2)
        gsl = slice(g * _HD, (g + 1) * _HD)
        in_maps.append({
            "xT": np.ascontiguousarray(x[b].T).astype(bf),
            "wq": np.ascontiguousarray(Wq[gsl, :].T).astype(bf),
            "wk": np.ascontiguousarray(Wk[gsl, :].T).astype(bf),
            "wv": np.ascontiguousarray(Wv[gsl, :].T).astype(bf),
            "wo": np.ascontiguousarray(Wo[:, gsl].T).astype(bf),
            "cc": ccm, "ss2": ss2m, "tri": trim, "tri2": tri2m,
        })
    return in_maps


def kernel(x, cos, sin, Wq, Wk, Wv, Wo):
    from concourse.bass_utils import run_bass_kernel_spmd

    x = np.asarray(x, dtype=np.float32)
    cos = np.asarray(cos, dtype=np.float32)
    sin = np.asarray(sin, dtype=np.float32)
    Wq = np.asarray(Wq, dtype=np.float32)
    Wk = np.asarray(Wk, dtype=np.float32)
    Wv = np.asarray(Wv, dtype=np.float32)
    Wo = np.asarray(Wo, dtype=np.float32)

    nc = _get_nc()
    in_maps = make_in_maps(x, cos, sin, Wq, Wk, Wv, Wo)
    res = run_bass_kernel_spmd(nc, in_maps, core_ids=list(range(8)))
    outs = [np.asarray(r_["out"]).astype(np.float32) for r_ in res.results]
    return np.stack([outs[2 * b] + outs[2 * b + 1] for b in range(_B)], axis=0)
